# revision 1
# baseline (speedup 1.0000x reference)
"""Causal self-attention (B=1, T=4096, C=1024, 16 heads x 64) on 8 TRN2 cores.

Sharding: tensor-parallel over heads. Core i computes heads (2i, 2i+1):
its slice of qkv, full causal attention for those heads, and the partial
output projection over its 128 y-dims. Host sums the 8 partial outputs.

Device layout (per core), everything f32r (fp32 bits, PE-rounded) so all
matmuls run at 1 cycle/row:
  xT   [1024, 4096]  x transposed (host-side) so contraction dim c is on partitions
  wqkv [1024, 384]   w_attn rows for q(2 heads),k,v transposed
  wp   [128, 1024]   w_proj columns for this core's 128 y-dims, transposed
  tri  [128, 384]    [lower-tri mask | ones | identity]
Attention computes S^T = K_chunk @ Q^T directly (so softmax sums come from an
appended ones-column in V via matmul), avoiding every transpose of S.
"""
import sys

sys.path.insert(0, "/opt/trn_rl_repo")

from contextlib import ExitStack

import numpy as np

import concourse.bacc as bacc
import concourse.mybir as mybir
import concourse.tile as tile
from concourse.bass_utils import run_bass_kernel_spmd

F32 = mybir.dt.float32
F32R = mybir.dt.float32r
EXP = mybir.ActivationFunctionType.Exp

P = 128
T = 4096
C = 1024
NH = 16
D = 64
NCORES = 8
HPC = NH // NCORES          # heads per core = 2
TB = 512                    # q-band width
NB = T // TB                # 8 bands
NKC = T // P                # 32 k-chunks
NCC = C // P                # 8 c-chunks
VP0_W = (NKC - 1) * 65 + 128    # [V0|1] blocks, stride 65
VP1_W = (NKC - 1) * 96 + 128    # [.|1|.|V1] blocks, stride 96, ones at +32
SCALE = 1.0 / (D ** 0.5)

_cache = {}


def _build():
    nc = bacc.Bacc("TRN2", target_bir_lowering=False, debug=False,
                   num_devices=NCORES)
    xT_d = nc.dram_tensor("xT", [C, T], F32R, kind="ExternalInput").ap()
    wqkv_d = nc.dram_tensor("wqkv", [C, 3 * P], F32R, kind="ExternalInput").ap()
    wp_d = nc.dram_tensor("wp", [P, C], F32R, kind="ExternalInput").ap()
    tri_d = nc.dram_tensor("tri", [P, 3 * P], F32R, kind="ExternalInput").ap()
    out_d = nc.dram_tensor("out", [T, C], F32, kind="ExternalOutput").ap()

    with tile.TileContext(nc) as tc:
        with ExitStack() as ctx:
            const = ctx.enter_context(tc.tile_pool(name="const", bufs=1))
            big = ctx.enter_context(tc.tile_pool(name="big", bufs=1))

            tri = const.tile([P, 3 * P], F32R)
            nc.sync.dma_start(tri[:], tri_d[:])
            # per-chunk weight DMAs: the first qkv matmul only needs chunk 0,
            # so band 0's x-load isn't queued behind the full 1.5MB transfer
            wqkv = const.tile([P, NCC * 3 * P], F32R)   # [p, c-chunk * 384]
            for cc in range(NCC):
                nc.sync.dma_start(
                    wqkv[:, cc * 3 * P:(cc + 1) * 3 * P],
                    wqkv_d[cc * P:(cc + 1) * P, :])
            wp = const.tile([P, C], F32R)

            qt = big.tile([P, T], F32R)     # Q^T, rows 0-63 h0, 64-127 h1
            kt = big.tile([P, T], F32R)
            vt = big.tile([P, T], F32R)
            vp0 = big.tile([P, VP0_W], F32R)
            vp1 = big.tile([P, VP1_W], F32R)
            yT = big.tile([P, T], F32R)
            # ones columns for the softmax-sum rows (strided: one col per chunk)
            nc.vector.tensor_copy(vp0[:, 64:64 + (NKC - 1) * 65 + 1:65],
                                  tri[:, P:P + NKC])
            nc.vector.tensor_copy(vp1[:, 32:32 + (NKC - 1) * 96 + 1:96],
                                  tri[:, P:P + NKC])

            # ---- fused per-band pipeline: qkv pipelined one band ahead ----
            with ExitStack() as c1:
                xpool = c1.enter_context(tc.tile_pool(name="xt", bufs=3))
                sexp_pool = c1.enter_context(tc.tile_pool(name="sexp", bufs=8))
                rc_pool = c1.enter_context(tc.tile_pool(name="rc", bufs=2))
                bcs_pool = c1.enter_context(tc.tile_pool(name="bcs", bufs=2))
                opool = c1.enter_context(tc.tile_pool(name="osb", bufs=2))
                # PSUM budget (8 banks): scores "sm" 2x[P,1024]=4, y 2x[P,512]=2,
                # aux 2x[P,512]=2 shared by qkv accum / V-transpose / bc / proj.
                ps_s = c1.enter_context(
                    tc.tile_pool(name="ps_s", bufs=2, space="PSUM"))
                ps_y = c1.enter_context(
                    tc.tile_pool(name="ps_y", bufs=1, space="PSUM"))
                ps_a = c1.enter_context(
                    tc.tile_pool(name="ps_a", bufs=2, space="PSUM"))
                xT3 = xT_d.rearrange("(c p) t -> p c t", p=P)

                def qkv_band(tb):
                    xt = xpool.tile([P, NCC * TB], F32R, name="xt")
                    hc = NCC // 2
                    for g in range(2):
                        nc.sync.dma_start(
                            xt[:, g * hc * TB:(g + 1) * hc * TB].rearrange(
                                "p (c t) -> p c t", c=hc),
                            xT3[:, g * hc:(g + 1) * hc,
                                tb * TB:(tb + 1) * TB])
                    for mt, dest in ((2, vt), (0, qt), (1, kt)):
                        ps = ps_a.tile([P, TB], F32, name="aux")
                        for cc in range(NCC):
                            nc.tensor.matmul(
                                ps[:],
                                wqkv[:, cc * 3 * P + mt * P:
                                     cc * 3 * P + (mt + 1) * P],
                                xt[:, cc * TB:(cc + 1) * TB],
                                start=(cc == 0), stop=(cc == NCC - 1))
                        nc.vector.tensor_copy(
                            dest[:, tb * TB:(tb + 1) * TB], ps[:])
                    for kc in range(4 * tb, 4 * tb + 4):
                        vps = ps_a.tile([P, TB], F32R, name="aux")
                        nc.tensor.transpose(
                            vps[:, 0:P], vt[:, kc * P:(kc + 1) * P],
                            tri[:, 2 * P:3 * P])
                        nc.vector.tensor_copy(
                            vp0[:, kc * 65:kc * 65 + 64], vps[:, 0:64])
                        nc.vector.tensor_copy(
                            vp1[:, kc * 96 + 64:kc * 96 + 128],
                            vps[:, 64:128])

                qkv_band(0)
                # wp is first needed at band 0's projection, ~30us in
                nc.sync.dma_start(wp[:], wp_d[:])
                qkv_band(1)
                pre = {}

                def scores_exp(tb, kc, col0):
                    s_ps = ps_s.tile([P, 2 * TB], F32, name="sm")
                    for h in range(HPC):
                        nc.tensor.matmul(
                            s_ps[:, h * TB + col0:(h + 1) * TB],
                            kt[h * D:(h + 1) * D, kc * P:(kc + 1) * P],
                            qt[h * D:(h + 1) * D,
                               tb * TB + col0:(tb + 1) * TB],
                            start=True, stop=True)
                    s_exp = sexp_pool.tile([P, 2 * TB], F32R, name="se")
                    if col0 >= 2 * P:
                        # diag chunk: skip the fully-masked columns
                        for h in range(HPC):
                            nc.scalar.activation(
                                s_exp[:, h * TB + col0:(h + 1) * TB],
                                s_ps[:, h * TB + col0:(h + 1) * TB],
                                EXP, scale=SCALE)
                    else:
                        nc.scalar.activation(s_exp[:], s_ps[:], EXP,
                                             scale=SCALE)
                    return s_exp

                for tb in range(NB):
                    # -- causal attention for this band --
                    nkc = 4 * tb + 4
                    y_ps = [ps_y.tile([P, TB], F32, name=f"y{h}")
                            for h in range(HPC)]
                    for kc in range(nkc):
                        col0 = (kc - 4 * tb) * P if kc >= 4 * tb else 0
                        if (tb, kc) in pre:
                            s_exp = pre.pop((tb, kc))
                        else:
                            s_exp = scores_exp(tb, kc, col0)
                        if kc == nkc - 1 and tb + 1 < NB:
                            # feed ACT through the band boundary: next band's
                            # first four chunks are always off-diagonal
                            for kc2 in range(4):
                                pre[(tb + 1, kc2)] = scores_exp(tb + 1, kc2, 0)
                        for h in range(HPC):
                            if kc >= 4 * tb:
                                nc.vector.tensor_mul(
                                    s_exp[:, h * TB + col0:h * TB + col0 + P],
                                    s_exp[:, h * TB + col0:h * TB + col0 + P],
                                    tri[:, 0:P])
                            lhs = (vp0[:, kc * 65:kc * 65 + P] if h == 0
                                   else vp1[:, kc * 96:kc * 96 + P])
                            nc.tensor.matmul(
                                y_ps[h][:, col0:TB], lhs,
                                s_exp[:, h * TB + col0:(h + 1) * TB],
                                start=(kc == 0), stop=(kc == nkc - 1))
                        if kc == 1 and tb + 2 < NB:
                            # band tb+2's qkv: PE fills ACT-bound slack
                            qkv_band(tb + 2)
                    # -- softmax denominators -> matmul broadcast -> yT --
                    rc = rc_pool.tile([P, TB], F32R, name="rc")
                    with nc.allow_low_precision(reason="f32r recip"):
                        nc.vector.reciprocal(rc[64:65, :], y_ps[0][64:65, :])
                        nc.vector.reciprocal(rc[32:33, :], y_ps[1][32:33, :])
                    bcs = bcs_pool.tile([P, TB], F32, name="bcs")
                    for h, (row, rows) in enumerate(
                            ((64, slice(0, 64)), (32, slice(64, 128)))):
                        bc = ps_a.tile([P, TB], F32, name="aux")
                        nc.tensor.matmul(bc[:],
                                         tri[row:row + 1, P:2 * P],
                                         rc[row:row + 1, :],
                                         start=True, stop=True)
                        nc.vector.tensor_copy(bcs[rows, :], bc[rows, :])
                    with nc.allow_low_precision(reason="f32r yT"):
                        nc.vector.tensor_mul(
                            yT[0:64, tb * TB:(tb + 1) * TB],
                            y_ps[0][0:64, :], bcs[0:64, :])
                        nc.vector.tensor_mul(
                            yT[64:128, tb * TB:(tb + 1) * TB],
                            y_ps[1][64:128, :], bcs[64:128, :])
                    # -- this band's 4 output-projection blocks --
                    for half_band in range(2):
                        osb = opool.tile([P, 2 * C], F32, name="osb")
                        for jj in range(2):
                            j = half_band * 2 + jj
                            t2 = 4 * tb + j
                            for half in range(2):
                                po = ps_a.tile([P, TB], F32, name="aux")
                                nc.tensor.matmul(
                                    po[:], yT[:, t2 * P:(t2 + 1) * P],
                                    wp[:, half * TB:(half + 1) * TB],
                                    start=True, stop=True)
                                nc.vector.tensor_copy(
                                    osb[:, jj * C + half * TB:
                                        jj * C + (half + 1) * TB], po[:])
                        nc.sync.dma_start(
                            out_d.rearrange("(g j p) o -> p g j o",
                                            j=2, p=P)[:, 2 * tb + half_band],
                            osb[:].rearrange("p (j o) -> p j o", j=2))

    nc.finalize()
    return nc


def _prep_inputs(x, w_attn, w_proj):
    xT = np.ascontiguousarray(x.reshape(T, C).T)          # [C, T]
    tri_m = (np.arange(P)[:, None] <= np.arange(P)[None, :]).astype(np.float32)
    tri = np.concatenate(
        [tri_m, np.ones((P, P), np.float32), np.eye(P, dtype=np.float32)],
        axis=1)
    in_maps = []
    for i in range(NCORES):
        hs = [HPC * i + j for j in range(HPC)]
        rows = []
        for base in (0, C, 2 * C):                         # q, k, v row blocks
            for h in hs:
                rows.append(w_attn[base + h * D:base + (h + 1) * D, :])
        wqkv = np.ascontiguousarray(np.concatenate(rows, axis=0).T)  # [C, 384]
        cols = np.concatenate([np.arange(h * D, (h + 1) * D) for h in hs])
        wp = np.ascontiguousarray(w_proj[:, cols].T)       # [128, C]
        in_maps.append({"xT": xT, "wqkv": wqkv, "wp": wp, "tri": tri})
    return in_maps


def kernel(x, w_attn, w_proj):
    x = np.asarray(x, dtype=np.float32)
    w_attn = np.asarray(w_attn, dtype=np.float32)
    w_proj = np.asarray(w_proj, dtype=np.float32)
    if "nc" not in _cache:
        _cache["nc"] = _build()
    nc = _cache["nc"]
    in_maps = _prep_inputs(x, w_attn, w_proj)
    res = run_bass_kernel_spmd(nc, in_maps, core_ids=list(range(NCORES)))
    out = np.zeros((T, C), np.float64)
    for i in range(NCORES):
        out += res.results[i]["out"].astype(np.float64)
    return out.astype(np.float32).reshape(1, T, C)



# revision 4
# speedup vs baseline: 1.1293x; 1.1293x over previous
"""Causal self-attention (B=1, T=4096, C=1024, 16 heads x 64) on 8 TRN2 cores.

Sharding: tensor-parallel over heads. Core i computes heads (2i, 2i+1):
its slice of qkv, full causal attention for those heads, and the partial
output projection over its 128 y-dims. Host sums the 8 partial outputs.

Engine plan (per core):
  PE   : qkv matmuls, scores K^T@Q, AV, proj              (~170us, critical)
  ACT  : exp on every causal score element (bf16 out)     (~145us)
  DVE  : psum->sbuf copies, diag masks, recip, y*1/den    (~45us)
  Pool : denominator partition_broadcast, proj staging    (~70us)
  DMA  : x in, out partials, V transposes via XBAR        (~60us)

Attention computes S^T = K_chunk @ Q^T (softmax sums via an appended
ones-column in V). exp results stream into a 40-chunk bf16 SBUF ring one
band ahead of AV, so AV never waits on ACT. All bf16 data; f32r weights;
f32 psum accumulation. Emission interleaves scores/AV/qkv/proj per slot
to keep PE busy.
"""
import sys

sys.path.insert(0, "/opt/trn_rl_repo")

from contextlib import ExitStack

import numpy as np
import ml_dtypes

import concourse.bacc as bacc
import concourse.mybir as mybir
import concourse.tile as tile
from concourse.bass_utils import run_bass_kernel_spmd

F32 = mybir.dt.float32
F32R = mybir.dt.float32r
BF16 = mybir.dt.bfloat16
EXP = mybir.ActivationFunctionType.Exp

P = 128
T = 4096
C = 1024
NH = 16
D = 64
NCORES = 8
HPC = NH // NCORES          # heads per core = 2
TB = 512                    # q-band width
NB = T // TB                # 8 bands
NCC = C // P                # 8 c-chunks
RING = 40                   # s_exp ring depth in chunks
VPS = 200                   # vp stride per k-chunk
SCALE = 1.0 / (D ** 0.5)

_cache = {}


def _gidx(b, kc):
    # global chunk index: band b holds chunks 0..4b+3
    return 2 * b * (b + 1) + kc


def _spread(items, nslots):
    """Partition items into nslots consecutive sublists, evenly."""
    out = []
    prev = 0
    for s in range(nslots):
        nxt = (s + 1) * len(items) // nslots
        out.append(items[prev:nxt])
        prev = nxt
    return out


def _build():
    nc = bacc.Bacc("TRN2", target_bir_lowering=False, debug=False,
                   num_devices=NCORES)
    xT_d = nc.dram_tensor("xT", [C, T], BF16, kind="ExternalInput").ap()
    wqkv_d = nc.dram_tensor("wqkv", [C, 3 * P], F32R, kind="ExternalInput").ap()
    wp_d = nc.dram_tensor("wp", [P, C], F32R, kind="ExternalInput").ap()
    tri_d = nc.dram_tensor("tri", [P, 2 * P], BF16, kind="ExternalInput").ap()
    out_d = nc.dram_tensor("out", [T, C], BF16, kind="ExternalOutput").ap()

    with tile.TileContext(nc) as tc:
        with ExitStack() as ctx:
            const = ctx.enter_context(tc.tile_pool(name="const", bufs=1))
            big = ctx.enter_context(tc.tile_pool(name="big", bufs=1))
            xpool = ctx.enter_context(tc.tile_pool(name="xt", bufs=3))
            rc_pool = ctx.enter_context(tc.tile_pool(name="rc", bufs=2))
            bcs_pool = ctx.enter_context(tc.tile_pool(name="bcs", bufs=2))
            opool = ctx.enter_context(tc.tile_pool(name="osb", bufs=2))
            # PSUM (8 banks): scores 2x[128,1024]=4, y 1x[128,1024]=2,
            # qkv/proj aux 2x[128,512]=2
            ps_s = ctx.enter_context(
                tc.tile_pool(name="ps_s", bufs=2, space="PSUM"))
            ps_y = ctx.enter_context(
                tc.tile_pool(name="ps_y", bufs=1, space="PSUM"))
            ps_q = ctx.enter_context(
                tc.tile_pool(name="ps_q", bufs=2, space="PSUM"))

            tri = const.tile([P, 2 * P], BF16)
            nc.sync.dma_start(tri[:], tri_d[:])
            wqkv = const.tile([P, NCC * 3 * P], F32R)   # [p, cc * 384]
            for cc in range(NCC):
                nc.sync.dma_start(
                    wqkv[:, cc * 3 * P:(cc + 1) * 3 * P],
                    wqkv_d[cc * P:(cc + 1) * P, :])
            wp = const.tile([P, C], F32R)

            qt = big.tile([P, T], BF16)     # rows 0:64 h0, 64:128 h1
            kt = big.tile([P, T], BF16)
            vt = big.tile([P, T], BF16)     # v-dims on partitions (pre-T)
            # vp: per k-chunk block of VPS cols:
            #   [V0^T(64) | ones(1)@64 | zeros | ones(1)@70 | zeros | V1^T(64)@134]
            # h0 lhsT = cols 0:65   -> y dims rows 0:64, sum row 64
            # h1 lhsT = cols 70:198 -> sum row 0, zeros rows 1:64, dims 64:128
            vp = big.tile([P, 32 * VPS], BF16)
            yT = big.tile([P, T], BF16)
            ring = big.tile([P, RING * 2 * TB], BF16)

            nc.gpsimd.memset(vp[:], 0.0)
            nc.gpsimd.memset(vp[:, 64:64 + 31 * VPS + 1:VPS], 1.0)
            nc.gpsimd.memset(vp[:, 70:70 + 31 * VPS + 1:VPS], 1.0)

            xT3 = xT_d.rearrange("(c p) t -> p c t", p=P)
            out_r = out_d.rearrange("(b j p) o -> p b j o", j=4, p=P)
            tri2 = tri[:].rearrange("p (h c) -> p h c", h=2)

            def dma_x(b, xt, split):
                ccs = 1 if split else 4
                for g in range(NCC // ccs):
                    nc.sync.dma_start(
                        xt[:, g * ccs * TB:(g + 1) * ccs * TB].rearrange(
                            "p (c t) -> p c t", c=ccs),
                        xT3[:, g * ccs:(g + 1) * ccs,
                            b * TB:(b + 1) * TB])

            def qkv_group(b, xt, mt, dest):
                ps = ps_q.tile([P, TB], F32, name="aux")
                mms = []
                for cc in range(NCC):
                    mms.append(lambda cc=cc, ps=ps: nc.tensor.matmul(
                        ps[:],
                        wqkv[:, cc * 3 * P + mt * P:cc * 3 * P + (mt + 1) * P],
                        xt[:, cc * TB:(cc + 1) * TB],
                        start=(cc == 0), stop=(cc == NCC - 1)))

                def fin(ps=ps, dest=dest, b=b, mt=mt):
                    with nc.allow_low_precision(reason="bf16 qkv"):
                        nc.vector.tensor_copy(
                            dest[:, b * TB:(b + 1) * TB], ps[:])
                    if mt == 2:  # v: transpose chunks into vp via DMA XBAR
                        for j in range(4):
                            kc = 4 * b + j
                            for h in range(HPC):
                                nc.sync.dma_start_transpose(
                                    vp[:, kc * VPS + (0 if h == 0 else 134):
                                       kc * VPS + (64 if h == 0 else 198)],
                                    vt[h * D:(h + 1) * D,
                                       kc * P:(kc + 1) * P])
                return mms, fin

            def qkv_units(b, xt):
                """PE units for band b's qkv; each unit = one matmul (the
                last of each group also runs the copy/transposes)."""
                units = []
                for mt, dest in ((2, vt), (0, qt), (1, kt)):
                    mms, fin = qkv_group(b, xt, mt, dest)
                    for i, mm in enumerate(mms):
                        if i == len(mms) - 1:
                            units.append(lambda mm=mm, fin=fin: (mm(), fin()))
                        else:
                            units.append(mm)
                return units

            def scores_chunk(b, kc):
                """Scores + exp into ring for chunk (b, kc)."""
                off = (kc - 4 * b) * P if kc >= 4 * b else 0
                s0 = (_gidx(b, kc) % RING) * 2 * TB
                s_ps = ps_s.tile([P, 2 * TB], F32, name="sm")
                for h in range(HPC):
                    nc.tensor.matmul(
                        s_ps[:, h * TB + off:(h + 1) * TB],
                        kt[h * D:(h + 1) * D, kc * P:(kc + 1) * P],
                        qt[h * D:(h + 1) * D, b * TB + off:(b + 1) * TB],
                        start=True, stop=True)
                if off == 0:
                    nc.scalar.activation(ring[:, s0:s0 + 2 * TB], s_ps[:],
                                         EXP, scale=SCALE)
                else:
                    for h in range(HPC):
                        nc.scalar.activation(
                            ring[:, s0 + h * TB + off:s0 + (h + 1) * TB],
                            s_ps[:, h * TB + off:(h + 1) * TB],
                            EXP, scale=SCALE)
                if kc >= 4 * b:
                    # mask the 128-col diagonal block of both heads (one op)
                    v = ring[:, s0:s0 + 2 * TB].rearrange(
                        "p (h c) -> p h c", h=2)[:, :, off:off + P]
                    with nc.allow_low_precision(reason="bf16 mask"):
                        nc.vector.tensor_mul(v, v, tri2[:, :, 0:P])

            def av_chunk(b, kc, y, c0, c1, start, stop):
                """AV for chunk (b, kc) over band cols [c0, c1)."""
                off = (kc - 4 * b) * P if kc >= 4 * b else 0
                lo = max(off, c0)
                if lo >= c1:
                    return
                s0 = (_gidx(b, kc) % RING) * 2 * TB
                nc.tensor.matmul(
                    y[0:65, lo:c1],
                    vp[:, kc * VPS:kc * VPS + 65],
                    ring[:, s0 + lo:s0 + c1],
                    start=start, stop=stop)
                nc.tensor.matmul(
                    y[0:P, TB + lo:TB + c1],
                    vp[:, kc * VPS + 70:kc * VPS + 198],
                    ring[:, s0 + TB + lo:s0 + TB + c1],
                    start=start, stop=stop)

            def band_drain(b, y, bcs, c0, c1):
                rc = rc_pool.tile([P, 2 * TB], F32, name="rc")
                with nc.allow_low_precision(reason="f32r recip"):
                    nc.vector.reciprocal(rc[64:65, c0:c1], y[64:65, c0:c1])
                    nc.vector.reciprocal(rc[0:1, TB + c0:TB + c1],
                                         y[0:1, TB + c0:TB + c1])
                nc.gpsimd.partition_broadcast(bcs[0:64, c0:c1],
                                              rc[64:65, c0:c1])
                nc.gpsimd.partition_broadcast(bcs[64:P, TB + c0:TB + c1],
                                              rc[0:1, TB + c0:TB + c1])
                with nc.allow_low_precision(reason="bf16 yT"):
                    nc.vector.tensor_mul(
                        yT[0:64, b * TB + c0:b * TB + c1],
                        y[0:64, c0:c1], bcs[0:64, c0:c1])
                    nc.vector.tensor_mul(
                        yT[64:P, b * TB + c0:b * TB + c1],
                        y[64:P, TB + c0:TB + c1], bcs[64:P, TB + c0:TB + c1])

            def proj_units(b, osb, js):
                """PE units for proj of band b, token-chunks js; emits the
                staging copy (Pool/DVE alternating) and the out DMA."""
                units = []
                for j in js:
                    t2 = 4 * b + j
                    for half in range(2):
                        def u(b=b, j=j, t2=t2, half=half, osb=osb):
                            po = ps_q.tile([P, TB], F32, name="aux")
                            nc.tensor.matmul(
                                po[:], yT[:, t2 * P:(t2 + 1) * P],
                                wp[:, half * TB:(half + 1) * TB],
                                start=True, stop=True)
                            eng = nc.gpsimd if (j + half) % 2 == 0 else nc.vector
                            with nc.allow_low_precision(reason="bf16 out"):
                                eng.tensor_copy(
                                    osb[:, j * C + half * TB:
                                        j * C + (half + 1) * TB], po[:])
                            if half == 1 and j % 2 == 1:
                                hb = j // 2
                                nc.sync.dma_start(
                                    out_r[:, b, 2 * hb:2 * hb + 2],
                                    osb[:, hb * 2 * C:(hb + 1) * 2 * C]
                                    .rearrange("p (j o) -> p j o", j=2))
                        units.append(u)
                return units

            # ---- prologue ----
            xt0 = xpool.tile([P, NCC * TB], BF16, name="xt")
            dma_x(0, xt0, split=True)
            q_units = {}
            u0 = qkv_units(0, xt0)          # order: v, q, k
            for u in u0[8:]:                # q, k groups first
                u()
            nc.sync.dma_start(wp[:], wp_d[:])
            scores_chunk(0, 0)
            for u in u0[0:8]:               # v group (+ transposes)
                u()
            scores_chunk(0, 1)
            xt1 = xpool.tile([P, NCC * TB], BF16, name="xt")
            dma_x(1, xt1, split=False)
            for u in qkv_units(1, xt1):
                u()
            scores_chunk(0, 2)
            scores_chunk(0, 3)

            # ---- main loop ----
            osb_prev = None
            for b in range(NB):
                nch = 4 * b + 4
                y = ps_y.tile([P, 2 * TB], F32, name="y")
                bcs = bcs_pool.tile([P, 2 * TB], F32, name="bcs")
                # assemble extra PE units for this band
                units = []
                if b + 2 < NB:
                    xt = xpool.tile([P, NCC * TB], BF16, name="xt")
                    dma_x(b + 2, xt, split=False)
                    units += qkv_units(b + 2, xt)
                osb = opool.tile([P, 4 * C], BF16, name="osb")
                if b >= 1:
                    units += proj_units(b - 1, osb_prev, [0, 1, 2, 3])
                scq = ([(b + 1, kc) for kc in range(4 * (b + 1) + 4)]
                       if b + 1 < NB else [])
                sc_sched = _spread(scq, nch)
                un_sched = _spread(units, nch)
                # front-load two scores chunks to hide band-start y drain
                if sc_sched and sc_sched[0]:
                    pass  # first slot's scores emitted before first AV below
                last = NB - 1
                for kc in range(nch):
                    for (sb, skc) in sc_sched[kc]:
                        scores_chunk(sb, skc)
                    if b == last:
                        # 4 independent 128-col accumulation groups so cols
                        # 0:256 can drain (and project) 2 chunks early
                        off = (kc - 4 * b) * P if kc >= 4 * b else 0
                        for g in range(4):
                            if off <= g * P:
                                av_chunk(b, kc, y, g * P, (g + 1) * P,
                                         start=(kc == 0),
                                         stop=(kc == nch - 4 + g))
                        if kc == nch - 3:
                            band_drain(b, y, bcs, 0, 2 * P)
                            for u in proj_units(b, osb, [0, 1]):
                                u()
                    else:
                        av_chunk(b, kc, y, 0, TB,
                                 start=(kc == 0), stop=(kc == nch - 1))
                    for u in un_sched[kc]:
                        u()
                if b == last:
                    band_drain(b, y, bcs, 2 * P, TB)
                    for u in proj_units(b, osb, [2, 3]):
                        u()
                else:
                    band_drain(b, y, bcs, 0, TB)
                osb_prev = osb

    nc.finalize()
    return nc


def _prep_inputs(x, w_attn, w_proj):
    xT = np.ascontiguousarray(
        x.reshape(T, C).T).astype(ml_dtypes.bfloat16)     # [C, T]
    tri_m = (np.arange(P)[:, None] <= np.arange(P)[None, :])
    tri = np.concatenate([tri_m, tri_m], axis=1).astype(ml_dtypes.bfloat16)
    in_maps = []
    for i in range(NCORES):
        hs = [HPC * i + j for j in range(HPC)]
        rows = []
        for base in (0, C, 2 * C):                         # q, k, v row blocks
            for h in hs:
                rows.append(w_attn[base + h * D:base + (h + 1) * D, :])
        wqkv = np.ascontiguousarray(np.concatenate(rows, axis=0).T)  # [C, 384]
        cols = np.concatenate([np.arange(h * D, (h + 1) * D) for h in hs])
        wp = np.ascontiguousarray(w_proj[:, cols].T)       # [128, C]
        in_maps.append({"xT": xT, "wqkv": wqkv, "wp": wp, "tri": tri})
    return in_maps


def kernel(x, w_attn, w_proj):
    x = np.asarray(x, dtype=np.float32)
    w_attn = np.asarray(w_attn, dtype=np.float32)
    w_proj = np.asarray(w_proj, dtype=np.float32)
    if "nc" not in _cache:
        _cache["nc"] = _build()
    nc = _cache["nc"]
    in_maps = _prep_inputs(x, w_attn, w_proj)
    res = run_bass_kernel_spmd(nc, in_maps, core_ids=list(range(NCORES)))
    out = np.zeros((T, C), np.float64)
    for i in range(NCORES):
        out += res.results[i]["out"].astype(np.float64)
    return out.astype(np.float32).reshape(1, T, C)
